# revision 1
# baseline (speedup 1.0000x reference)
"""Trainium2 Bass kernel for ContextualAttentionBlock.

Sharding: 8 cores, core c -> (batch b = c//2, query-half qh = c%2).
Each core computes, for its batch's 1024-token attention window:
  K/V projections for all 1024 tokens, Q for its 512 queries, RoPE,
  attention, out-proj, residual+RMSNorm1 -> h1 (512 tokens),
then SwiGLU FFN + residual + RMSNorm2 for 2048 tokens
  (512 attention-part tokens + 1536 "rest" tokens that skip attention).
All activations are kept feature-major ([feature, token]) so every matmul
contracts over the partition dim.  Matmuls run as float32r (TF32).
No collectives; the host shards inputs and reassembles the output.
"""

import numpy as np

import concourse.bass as bass
import concourse.tile as tile
from concourse import bacc, mybir
from concourse.bass_utils import run_bass_kernel_spmd

F32 = mybir.dt.float32
F32R = mybir.dt.float32r
AF = mybir.ActivationFunctionType
OP = mybir.AluOpType

WIDTH = 1024
NT = 8              # width tiles of 128
HEADS = 16
HDIM = 64
LWIN = 1024         # attention window
LQ = 512            # queries per core
HID = 4096
NH = 32             # hidden tiles of 128
REST = 1536         # rest tokens per core
CH = 512            # ffn token chunk
NCH = 4             # ffn chunks: LQ + REST = 2048 = 4*512
TOUT = LQ + REST
EPS = 1e-6
ROPE_BASE = 10000.0
N_CORES = 8


def _r(ap):
    return ap.bitcast(mybir.dt.float32r)


def _emit(tc, A, out_ap):
    nc = tc.nc
    mm = nc.tensor.matmul

    xw_r = A["xw_t"].rearrange("(a p) t -> a p t", p=128)
    xq_r = A["xq_t"].rearrange("(a p) t -> a p t", p=128)
    xr_r = A["xr_t"].rearrange("(a p) t -> a p t", p=128)
    wq_r = A["wq_t"].rearrange("(a p) m -> p a m", p=128)
    wk_r = A["wk_t"].rearrange("(a p) m -> p a m", p=128)
    wv_r = A["wv_t"].rearrange("(a p) m -> p a m", p=128)
    wo_r = A["wo_t"].rearrange("(a p) m -> p a m", p=128)
    wg_r = A["wg_t"].rearrange("(a p) m -> p a m", p=128)
    wu_r = A["wu_t"].rearrange("(a p) m -> p a m", p=128)
    wd_r = A["wd_t"].rearrange("(a p) m -> p a m", p=128)
    out_r = out_ap.rearrange("(a p) t -> a p t", p=128)

    with tc.tile_pool(name="pc", bufs=1) as pc:
        cq = pc.tile([128, LQ], F32R, name="cq")
        nc.sync.dma_start(cq, A["cos_q"])
        sq = pc.tile([128, LQ], F32R, name="sq")
        nc.sync.dma_start(sq, A["sin_q"])
        ck = pc.tile([128, LWIN], F32R, name="ck")
        nc.sync.dma_start(ck, A["cos_k"])
        sk = pc.tile([128, LWIN], F32R, name="sk")
        nc.sync.dma_start(sk, A["sin_k"])
        g1 = pc.tile([128, NT], F32R, name="g1")
        nc.sync.dma_start(g1, A["g1"])
        g2 = pc.tile([128, NT], F32R, name="g2")
        nc.sync.dma_start(g2, A["g2"])
        onesF = pc.tile([128, 128], F32, name="onesF")
        nc.vector.memset(onesF, 1.0)
        onesK = pc.tile([128, 1], F32R, name="onesK")
        nc.vector.tensor_copy(onesK, onesF[:, 0:1])
        ones1 = pc.tile([1, 128], F32R, name="ones1")
        nc.vector.tensor_copy(ones1, onesF[0:1, :])
        eps1 = pc.tile([1, 1], F32, name="eps1")
        nc.vector.memset(eps1, EPS)
        ones64 = pc.tile([1, 64], F32R, name="ones64")
        nc.vector.tensor_copy(ones64, onesF[0:1, 0:64])
        h1 = [pc.tile([128, LQ], F32R, name=f"h1_{m}", tag=f"h1_{m}") for m in range(NT)]
        xq = [pc.tile([128, LQ], F32R, name=f"xq_{k}", tag=f"xq_{k}") for k in range(NT)]
        for k in range(NT):
            nc.sync.dma_start(xq[k], xq_r[k])

        # ---------------- Stage A: attention ----------------
        with tc.tile_pool(name="pb", bufs=1) as pb:
            ao = [pb.tile([128, LQ], F32R, name=f"ao_{i}", tag=f"ao{i}") for i in range(NT)]
            s_sb = [pb.tile([128, LQ], F32R, name=f"s_{m}", tag=f"s{m}") for m in range(NT)]
            with tc.tile_pool(name="pa", bufs=1) as pa, \
                 tc.tile_pool(name="wa", bufs=1) as wa, \
                 tc.tile_pool(name="psa", bufs=1, space="PSUM") as psa:
                xw = [pa.tile([128, LWIN], F32R, name=f"xw_{k}", tag=f"xw{k}") for k in range(NT)]
                for k in range(NT):
                    nc.sync.dma_start(xw[k], xw_r[k])

                def rope_from_psum(ps, dest, cos, sin, scratch, T):
                    # per 64-row head block: rows b..b+32 = even dims E,
                    # rows b+32..b+64 = odd dims O (head-contiguous perm).
                    # dest[b:b+32]    = E*cos - O*sin
                    # dest[b+32:b+64] = O*cos + E*sin
                    # cos/sin rows repeat every 32 (freq = row % 32), so the
                    # SBUF operand can always share the output's base partition.
                    for b in (0, 64):
                        e_ps, o_ps = ps[b:b + 32, :], ps[b + 32:b + 64, :]
                        nc.vector.tensor_mul(scratch[b + 32:b + 64, :], e_ps,
                                             sin[b + 32:b + 64, :])  # E*sin
                        nc.vector.tensor_mul(scratch[b:b + 32, :], o_ps,
                                             sin[b:b + 32, :])       # O*sin
                        nc.vector.tensor_mul(dest[b:b + 32, :], e_ps, cos[b:b + 32, :])
                        nc.vector.tensor_sub(dest[b:b + 32, :], dest[b:b + 32, :],
                                             scratch[b:b + 32, :])
                        nc.vector.tensor_mul(dest[b + 32:b + 64, :], o_ps,
                                             cos[b + 32:b + 64, :])
                        nc.vector.tensor_add(dest[b + 32:b + 64, :],
                                             dest[b + 32:b + 64, :],
                                             scratch[b + 32:b + 64, :])

                for g in range(4):
                    # --- Q projection + RoPE (tiles 2g, 2g+1; heads 4g..4g+3) ---
                    q2 = []
                    for side, m in ((0, 2 * g), (1, 2 * g + 1)):
                        wb = wa.tile([128, NT, 128], F32R, name=f"wqb_{g}_{side}", tag="wqk", bufs=2)
                        nc.sync.dma_start(wb, wq_r[:, :, m * 128:(m + 1) * 128])
                        ps = psa.tile([128, LQ], F32, name=f"qps_{g}_{side}", tag="proj", bufs=2)
                        for k in range(NT):
                            mm(ps, _r(wb[:, k, :]), _r(xq[k]), start=(k == 0), stop=(k == NT - 1))
                        qt = pa.tile([128, LQ], F32R, name=f"q_{g}_{side}",
                                     tag=("q0" if side == 0 else "q1"), bufs=2)
                        qs = pa.tile([128, LQ], F32R, name=f"qs_{g}_{side}", tag="rs", bufs=2)
                        rope_from_psum(ps, qt, cq, sq, qs, LQ)
                        q2.append(qt)

                    # --- K projection + RoPE ---
                    k2 = []
                    for side, m in ((0, 2 * g), (1, 2 * g + 1)):
                        wb = wa.tile([128, NT, 128], F32R, name=f"wkb_{g}_{side}", tag="wqk", bufs=2)
                        nc.sync.dma_start(wb, wk_r[:, :, m * 128:(m + 1) * 128])
                        kt_sb = pa.tile([128, LWIN], F32R, name=f"k_{g}_{side}",
                                        tag=("k0" if side == 0 else "k1"), bufs=2)
                        for ch in range(2):
                            ps = psa.tile([128, 512], F32, name=f"kps_{g}_{side}_{ch}", tag="proj", bufs=2)
                            for k in range(NT):
                                mm(ps, _r(wb[:, k, :]), _r(xw[k][:, ch * 512:(ch + 1) * 512]),
                                   start=(k == 0), stop=(k == NT - 1))
                            ks = pa.tile([128, 512], F32R, name=f"ks_{g}_{side}_{ch}", tag="rs", bufs=2)
                            rope_from_psum(ps, kt_sb[:, ch * 512:(ch + 1) * 512],
                                           ck[:, ch * 512:(ch + 1) * 512],
                                           sk[:, ch * 512:(ch + 1) * 512], ks, 512)
                        k2.append(kt_sb)

                    # --- V projection (token-major, 65-col per head with ones col) ---
                    wvb = wa.tile([128, NT, 256], F32R, name=f"wvb_{g}", tag="wv", bufs=1)
                    nc.sync.dma_start(wvb, wv_r[:, :, g * 256:(g + 1) * 256])
                    vg = [pa.tile([128, 4 * 65], F32R, name=f"v_{g}_{kt}", tag=f"v{kt}", bufs=2)
                          for kt in range(NT)]
                    for kt in range(NT):
                        psv = psa.tile([128, 256], F32, name=f"vps_{g}_{kt}", tag="proj", bufs=2)
                        for k in range(NT):
                            mm(psv, _r(xw[k][:, kt * 128:(kt + 1) * 128]), _r(wvb[:, k, :]),
                               start=(k == 0), stop=(k == NT - 1))
                        v3 = vg[kt].rearrange("p (h c) -> p h c", c=65)
                        nc.vector.tensor_copy(v3[:, :, 64:65],
                                              onesF[:, 0:1].unsqueeze(1).broadcast_to([128, 4, 1]))
                        nc.vector.tensor_copy(v3[:, :, 0:64], psv.rearrange("p (h c) -> p h c", c=64))

                    # --- attention per head pair ---
                    for p2 in range(2):
                        vac = [psa.tile([65, 512], F32, name=f"vac_{g}_{p2}_{jj}", tag="vac", bufs=4)
                               for jj in range(2)]
                        for kt in range(NT):
                            for jj in range(2):
                                j = 2 * p2 + jj
                                sc = psa.tile([128, 512], F32, name=f"sc_{g}_{p2}_{kt}_{jj}",
                                              tag="sc", bufs=2)
                                mm(sc, _r(k2[p2][64 * jj:64 * (jj + 1), kt * 128:(kt + 1) * 128]),
                                   _r(q2[p2][64 * jj:64 * (jj + 1), :]),
                                   start=True, stop=True, tile_position=(64 * jj, 0))
                                at = pa.tile([128, 512], F32R, name=f"at_{g}_{p2}_{kt}_{jj}",
                                             tag="at", bufs=3)
                                nc.scalar.activation(at, sc, AF.Exp, scale=0.125)
                                mm(vac[jj], _r(vg[kt][:, j * 65:(j + 1) * 65]), _r(at),
                                   start=(kt == 0), stop=(kt == NT - 1))
                        # normalize pair -> attention out tile i (heads 2i, 2i+1)
                        rr = pa.tile([1, 1024], F32R, name=f"rr_{g}_{p2}", tag="rr", bufs=2)
                        nc.scalar.copy(rr[0:1, 0:512], vac[0][64:65, :])
                        nc.scalar.copy(rr[0:1, 512:1024], vac[1][64:65, :])
                        with nc.allow_low_precision(reason="tf32 softmax denom"):
                            nc.vector.reciprocal(rr, rr)
                        bc0 = psa.tile([64, 512], F32, name=f"bca_{g}_{p2}_0", tag="vac", bufs=4)
                        mm(bc0, _r(ones64), _r(rr[0:1, 0:512]))
                        bc1 = psa.tile([64, 512], F32, name=f"bca_{g}_{p2}_1", tag="vac", bufs=4)
                        mm(bc1, _r(ones64), _r(rr[0:1, 512:1024]))
                        bcs = pa.tile([128, 512], F32R, name=f"bcs_{g}_{p2}", tag="bcs", bufs=1)
                        nc.scalar.copy(bcs[0:64, :], bc0)
                        nc.scalar.copy(bcs[64:128, :], bc1)
                        i = 2 * g + p2
                        nc.vector.tensor_mul(ao[i][0:64, :], vac[0][0:64, :], bcs[0:64, :])
                        nc.vector.tensor_mul(ao[i][64:128, :], vac[1][0:64, :], bcs[64:128, :])

                # ---------------- Stage B: out-proj + RMSNorm1 -> h1 ----------------
                ssp = psa.tile([1, 512], F32, name="ssp", tag="sc", bufs=2)
                for m in range(NT):
                    wb = wa.tile([128, NT, 128], F32R, name=f"wob_{m}", tag="wqk", bufs=2)
                    nc.sync.dma_start(wb, wo_r[:, :, m * 128:(m + 1) * 128])
                    yp = psa.tile([128, LQ], F32, name=f"yps_{m}", tag="proj", bufs=2)
                    for k in range(NT):
                        mm(yp, _r(wb[:, k, :]), _r(ao[k]), start=(k == 0), stop=(k == NT - 1))
                    nc.vector.tensor_add(s_sb[m], xq[m], yp)
                    sqt = pa.tile([128, LQ], F32R, name=f"sq1_{m}", tag="sq", bufs=2)
                    nc.vector.tensor_mul(sqt, s_sb[m], s_sb[m])
                    mm(ssp, _r(onesK), _r(sqt), start=(m == 0), stop=(m == NT - 1))
                row = pa.tile([1, 512], F32R, name="row1", tag="row", bufs=2)
                nc.scalar.activation(row, ssp, AF.Sqrt, scale=1.0 / WIDTH, bias=eps1)
                with nc.allow_low_precision(reason="tf32 rstd"):
                    nc.vector.reciprocal(row, row)
                bcn = psa.tile([128, 512], F32, name="bcn", tag="vac", bufs=4)
                mm(bcn, _r(ones1), _r(row))
                for m in range(NT):
                    nc.vector.scalar_tensor_tensor(h1[m], s_sb[m], g1[:, m:m + 1], bcn,
                                                   op0=OP.mult, op1=OP.mult)

        # ---------------- Stage C: SwiGLU FFN + RMSNorm2 ----------------
        with tc.tile_pool(name="pcn", bufs=1) as pcn, \
             tc.tile_pool(name="wc", bufs=1) as wc, \
             tc.tile_pool(name="psc", bufs=1, space="PSUM") as psc:
            F_t = [pcn.tile([128, CH], F32R, name=f"F_{h}", tag=f"F{h}") for h in range(NH)]
            for c in range(NCH):
                if c == 0:
                    tin = h1
                else:
                    tin = [pcn.tile([128, CH], F32R, name=f"tin_{c}_{k}", tag=f"tin{k}", bufs=2)
                           for k in range(NT)]
                    for k in range(NT):
                        nc.sync.dma_start(tin[k], xr_r[k][:, (c - 1) * CH:c * CH])
                for hm in range(NH):
                    wgb = wc.tile([128, NT, 128], F32R, name=f"wgb_{c}_{hm}", tag="wgu", bufs=4)
                    nc.sync.dma_start(wgb, wg_r[:, :, hm * 128:(hm + 1) * 128])
                    wub = wc.tile([128, NT, 128], F32R, name=f"wub_{c}_{hm}", tag="wgu", bufs=4)
                    nc.sync.dma_start(wub, wu_r[:, :, hm * 128:(hm + 1) * 128])
                    gp = psc.tile([128, CH], F32, name=f"gp_{c}_{hm}", tag="gu", bufs=3)
                    up = psc.tile([128, CH], F32, name=f"up_{c}_{hm}", tag="gu", bufs=3)
                    for k in range(NT):
                        mm(gp, _r(wgb[:, k, :]), _r(tin[k]), start=(k == 0), stop=(k == NT - 1))
                    for k in range(NT):
                        mm(up, _r(wub[:, k, :]), _r(tin[k]), start=(k == 0), stop=(k == NT - 1))
                    sg = pcn.tile([128, CH], F32R, name=f"sg_{c}_{hm}", tag="sg", bufs=2)
                    nc.scalar.activation(sg, gp, AF.Silu)
                    nc.vector.tensor_mul(F_t[hm], sg, up)
                s2l = [pcn.tile([128, CH], F32R, name=f"s2_{c}_{m}", tag=f"s2_{m}") for m in range(NT)]
                ssp2 = psc.tile([1, CH], F32, name=f"ssp2_{c}", tag="st", bufs=1)
                for m in range(NT):
                    dp = psc.tile([128, CH], F32, name=f"dp_{c}_{m}", tag="d", bufs=2)
                    for hh in range(2):
                        wdb = wc.tile([128, 16, 128], F32R, name=f"wdb_{c}_{m}_{hh}", tag="wd", bufs=2)
                        nc.sync.dma_start(wdb, wd_r[:, hh * 16:(hh + 1) * 16, m * 128:(m + 1) * 128])
                        for h2 in range(16):
                            hm = hh * 16 + h2
                            mm(dp, _r(wdb[:, h2, :]), _r(F_t[hm]), start=(hm == 0), stop=(hm == NH - 1))
                    nc.vector.tensor_add(s2l[m], tin[m], dp)
                    sqt = pcn.tile([128, CH], F32R, name=f"sq2_{c}_{m}", tag="sg", bufs=2)
                    nc.vector.tensor_mul(sqt, s2l[m], s2l[m])
                    mm(ssp2, _r(onesK), _r(sqt), start=(m == 0), stop=(m == NT - 1))
                row2 = pcn.tile([1, CH], F32R, name=f"row2_{c}", tag="row2", bufs=2)
                nc.scalar.activation(row2, ssp2, AF.Sqrt, scale=1.0 / WIDTH, bias=eps1)
                with nc.allow_low_precision(reason="tf32 rstd"):
                    nc.vector.reciprocal(row2, row2)
                bc2 = psc.tile([128, CH], F32, name=f"bc2_{c}", tag="d", bufs=2)
                mm(bc2, _r(ones1), _r(row2))
                for m in range(NT):
                    ot = pcn.tile([128, CH], F32R, name=f"ot_{c}_{m}", tag="ot", bufs=2)
                    nc.vector.scalar_tensor_tensor(ot, s2l[m], g2[:, m:m + 1], bc2,
                                                   op0=OP.mult, op1=OP.mult)
                    nc.sync.dma_start(out_r[m][:, c * CH:(c + 1) * CH], ot)


_INPUT_SPECS = [
    ("xw_t", [WIDTH, LWIN]),
    ("xq_t", [WIDTH, LQ]),
    ("xr_t", [WIDTH, REST]),
    ("wq_t", [WIDTH, WIDTH]),
    ("wk_t", [WIDTH, WIDTH]),
    ("wv_t", [WIDTH, WIDTH]),
    ("wo_t", [WIDTH, WIDTH]),
    ("wg_t", [WIDTH, HID]),
    ("wu_t", [WIDTH, HID]),
    ("wd_t", [HID, WIDTH]),
    ("cos_q", [128, LQ]),
    ("sin_q", [128, LQ]),
    ("cos_k", [128, LWIN]),
    ("sin_k", [128, LWIN]),
    ("g1", [128, NT]),
    ("g2", [128, NT]),
]


def build_program(reps=1):
    nc = bacc.Bacc("TRN2", target_bir_lowering=False, debug=False, num_devices=N_CORES)
    A = {name: nc.dram_tensor(name, shape, F32R, kind="ExternalInput").ap()
         for name, shape in _INPUT_SPECS}
    out_ap = nc.dram_tensor("out_t", [WIDTH, TOUT], F32R, kind="ExternalOutput").ap()
    with tile.TileContext(nc) as tc:
        for _ in range(reps):
            _emit(tc, A, out_ap)
    nc.compile()
    return nc


def make_in_maps(x, w_qkv, w_out, g_norm1, g_norm2, w_gate, w_up, w_down):
    f32 = np.float32
    x = np.asarray(x, f32)
    w_qkv = np.asarray(w_qkv, f32)
    # head-contiguous de-interleave: within head h, even dims first then odd:
    # new row h*64+j -> old h*64+2j ; new row h*64+32+j -> old h*64+2j+1
    perm = np.empty(WIDTH, np.int64)
    for h in range(HEADS):
        j = np.arange(32)
        perm[h * 64 + j] = h * 64 + 2 * j
        perm[h * 64 + 32 + j] = h * 64 + 2 * j + 1
    wq = w_qkv[0:WIDTH][perm]
    wk = w_qkv[WIDTH:2 * WIDTH][perm]
    wv = w_qkv[2 * WIDTH:3 * WIDTH]

    inv_freq = (1.0 / (ROPE_BASE ** (np.arange(0, HDIM, 2, dtype=np.float64) / HDIM)))

    def tab(pos):
        fr = np.outer(inv_freq, pos.astype(np.float64))  # [32, T]
        return (np.tile(np.cos(fr), (4, 1)).astype(f32),
                np.tile(np.sin(fr), (4, 1)).astype(f32))

    cos_k, sin_k = tab(np.arange(LWIN))
    common = {
        "wq_t": np.ascontiguousarray(wq.T),
        "wk_t": np.ascontiguousarray(wk.T),
        "wv_t": np.ascontiguousarray(wv.T),
        "wo_t": np.ascontiguousarray(np.asarray(w_out, f32).T),
        "wg_t": np.ascontiguousarray(np.asarray(w_gate, f32).T),
        "wu_t": np.ascontiguousarray(np.asarray(w_up, f32).T),
        "wd_t": np.ascontiguousarray(np.asarray(w_down, f32).T),
        "cos_k": cos_k,
        "sin_k": sin_k,
        "g1": np.ascontiguousarray(np.asarray(g_norm1, f32).reshape(NT, 128).T),
        "g2": np.ascontiguousarray(np.asarray(g_norm2, f32).reshape(NT, 128).T),
    }
    in_maps = []
    for c in range(N_CORES):
        b, qh = c // 2, c % 2
        cos_q, sin_q = tab(np.arange(qh * LQ, (qh + 1) * LQ))
        m = dict(common)
        m["xw_t"] = np.ascontiguousarray(x[b, :LWIN].T)
        m["xq_t"] = np.ascontiguousarray(x[b, qh * LQ:(qh + 1) * LQ].T)
        m["xr_t"] = np.ascontiguousarray(x[b, LWIN + qh * REST:LWIN + (qh + 1) * REST].T)
        m["cos_q"] = cos_q
        m["sin_q"] = sin_q
        in_maps.append(m)
    return in_maps


def assemble_output(results):
    out = np.empty((4, 4096, WIDTH), np.float32)
    for c in range(N_CORES):
        b, qh = c // 2, c % 2
        o = results[c]["out_t"]
        out[b, qh * LQ:(qh + 1) * LQ] = o[:, :LQ].T
        out[b, LWIN + qh * REST:LWIN + (qh + 1) * REST] = o[:, LQ:].T
    return out


_CACHE = {}


def kernel(x, w_qkv, w_out, g_norm1, g_norm2, w_gate, w_up, w_down):
    if "nc" not in _CACHE:
        _CACHE["nc"] = build_program()
    nc = _CACHE["nc"]
    in_maps = make_in_maps(x, w_qkv, w_out, g_norm1, g_norm2, w_gate, w_up, w_down)
    res = run_bass_kernel_spmd(nc, in_maps, list(range(N_CORES))).results
    return assemble_output(res)



# revision 13
# speedup vs baseline: 1.3720x; 1.3720x over previous
"""Trainium2 Bass kernel for ContextualAttentionBlock.

Sharding: 8 cores, core c -> (batch b = c//2, query-half qh = c%2).
Each core computes, for its batch's 1024-token attention window:
  K/V projections for all 1024 tokens, Q for its 512 queries, RoPE,
  attention, out-proj, residual+RMSNorm1 -> h1 (512 tokens),
then SwiGLU FFN + residual + RMSNorm2 for 2048 tokens
  (512 attention-part tokens + 1536 "rest" tokens that skip attention).

All matmul operands are bf16 (fp32 PSUM accumulation); residual/norm
statistics stay fp32.  Weights are host-packed into per-DMA contiguous
blobs so each weight load is one large descriptor-friendly transfer.
RoPE is computed as  x*cos + (P@x)*sin  where P is a constant +-1
permutation matrix applied on the tensor engine, so the vector engine
only does 3 full-width ops per tile.
No collectives; the host shards inputs and reassembles the output.
"""

import numpy as np
import ml_dtypes

import concourse.bass as bass
import concourse.tile as tile
from concourse import bacc, mybir
from concourse.bass_utils import run_bass_kernel_spmd

F32 = mybir.dt.float32
BF16 = mybir.dt.bfloat16
AF = mybir.ActivationFunctionType
OP = mybir.AluOpType

WIDTH = 1024
NT = 8              # width tiles of 128
HEADS = 16
HDIM = 64
LWIN = 1024         # attention window
LQ = 512            # queries per core
HID = 4096
NH = 32             # hidden tiles of 128
REST = 1536         # rest tokens per core
TOUT = LQ + REST    # 2048 tokens through the FFN per core
NSUB = 4            # 4 token sub-tiles of 512: [h1, xr0, xr1, xr2]
EPS = 1e-6
ROPE_BASE = 10000.0
N_CORES = 8
BF = ml_dtypes.bfloat16


def _emit(tc, A, out_ap):
    nc = tc.nc
    mm = nc.tensor.matmul
    wgu_r = A["wgu_p"].rearrange("p h g k j -> h p g k j")
    wd_r = A["wd_p"].rearrange("p m h j -> m p h j")
    out_r = out_ap.rearrange("p m t -> m p t")

    with tc.tile_pool(name="pc", bufs=1) as pc:
        onesb = pc.tile([128, 128], BF16, name="onesb")
        nc.sync.dma_start(onesb, A["ones_b"])
        onesK = onesb[:, 0:1]       # [128,1]
        ones1 = onesb[0:1, :]       # [1,128]
        ones64 = onesb[0:1, 0:64]   # [1,64]
        eps1 = pc.tile([1, 1], F32, name="eps1")
        nc.vector.memset(eps1, EPS)
        g1 = pc.tile([128, NT], F32, name="g1")
        nc.sync.dma_start(g1, A["g1"])
        g2 = pc.tile([128, NT], F32, name="g2")
        nc.sync.dma_start(g2, A["g2"])
        h1f = [pc.tile([128, LQ], F32, name=f"h1f_{m}", tag=f"h1f{m}") for m in range(NT)]
        h1b = [pc.tile([128, LQ], BF16, name=f"h1b_{m}", tag=f"h1b{m}") for m in range(NT)]

        # ---------------- Stage A: attention ----------------
        with tc.tile_pool(name="pa", bufs=1) as pa, \
             tc.tile_pool(name="wa", bufs=1) as wa, \
             tc.tile_pool(name="psa", bufs=1, space="PSUM") as psa:
            wq_r = A["wq_p"].rearrange("p k (gg ss) j -> gg p k ss j", ss=2)
            wk_r = A["wk_p"].rearrange("p k (gg ss) j -> gg p k ss j", ss=2)
            xq_b = pa.tile([128, NT, LQ], BF16, name="xq_b")
            nc.sync.dma_start(xq_b, A["xq_b"])
            prot = wa.tile([128, 128], BF16, name="prot")
            nc.sync.dma_start(prot, A["rope_p"])
            cq = pa.tile([128, LQ], F32, name="cq")
            nc.sync.dma_start(cq, A["cos_q"])
            sq = pa.tile([128, LQ], F32, name="sq")
            nc.sync.dma_start(sq, A["sin_q"])
            xw_b = pa.tile([128, NT, LWIN], BF16, name="xw_b")
            nc.sync.dma_start(xw_b, A["xw_b"])
            ck = pa.tile([128, LWIN], F32, name="ck")
            nc.sync.dma_start(ck, A["cos_k"])
            sk = pa.tile([128, LWIN], F32, name="sk")
            nc.sync.dma_start(sk, A["sin_k"])
            wv_sb = wa.tile([128, NT, WIDTH], BF16, name="wv_sb")
            nc.sync.dma_start(wv_sb, A["wv_p"])
            wo_sb = wa.tile([128, NT, WIDTH], BF16, name="wo_sb")
            nc.sync.dma_start(wo_sb, A["wo_p"])
            xq_f = pa.tile([128, NT, LQ], F32, name="xq_f")
            nc.sync.dma_start(xq_f, A["xq_f"])

            ao = [pa.tile([128, LQ], BF16, name=f"ao_{i}", tag=f"ao{i}") for i in range(NT)]
            s_sb = [pa.tile([128, LQ], F32, name=f"s_{m}", tag=f"s{m}") for m in range(NT)]

            def rope(dest, raw_sb, rot_ps, cos_t, sin_t, w):
                # dest = raw*cos + (P@raw)*sin, all [128, w] full-width ops
                t1 = pa.tile([128, w], F32, name="rt1", tag="rt1", bufs=2)
                nc.vector.tensor_mul(t1, raw_sb, cos_t)
                t2 = pa.tile([128, w], F32, name="rt2", tag="rt2", bufs=2)
                nc.vector.tensor_mul(t2, rot_ps, sin_t)
                nc.vector.tensor_add(dest, t1, t2)

            for g in range(4):
                # --- Q projection + RoPE (tiles 2g, 2g+1; heads 4g..4g+3) ---
                wqg = wa.tile([128, NT, 2, 128], BF16, name=f"wqg_{g}", tag="wqg", bufs=2)
                nc.sync.dma_start(wqg, wq_r[g])
                wkg = wa.tile([128, NT, 2, 128], BF16, name=f"wkg_{g}", tag="wkg", bufs=2)
                nc.sync.dma_start(wkg, wk_r[g])
                q2 = []
                for side in (0, 1):
                    ps = psa.tile([128, LQ], F32, name=f"qps_{g}_{side}", tag="proj", bufs=3)
                    for k in range(NT):
                        mm(ps, wqg[:, k, side, :], xq_b[:, k, :],
                           start=(k == 0), stop=(k == NT - 1))
                    qraw = pa.tile([128, LQ], BF16, name=f"qraw_{g}_{side}", tag="raw", bufs=2)
                    nc.scalar.copy(qraw, ps)
                    rot = psa.tile([128, LQ], F32, name=f"qrot_{g}_{side}", tag="proj", bufs=3)
                    mm(rot, prot, qraw)
                    qt = pa.tile([128, LQ], BF16, name=f"q_{g}_{side}",
                                 tag=("q0" if side == 0 else "q1"), bufs=2)
                    rope(qt, qraw, rot, cq, sq, LQ)
                    q2.append(qt)

                # --- K projection + RoPE ---
                k2 = []
                for side in (0, 1):
                    kt_sb = pa.tile([128, LWIN], BF16, name=f"k_{g}_{side}",
                                    tag=("k0" if side == 0 else "k1"), bufs=2)
                    for ch in range(2):
                        ps = psa.tile([128, 512], F32, name=f"kps_{g}_{side}_{ch}",
                                      tag="proj", bufs=3)
                        for k in range(NT):
                            mm(ps, wkg[:, k, side, :], xw_b[:, k, ch * 512:(ch + 1) * 512],
                               start=(k == 0), stop=(k == NT - 1))
                        kraw = pa.tile([128, 512], BF16, name=f"kraw_{g}_{side}_{ch}",
                                       tag="raw", bufs=2)
                        nc.scalar.copy(kraw, ps)
                        rot = psa.tile([128, 512], F32, name=f"krot_{g}_{side}_{ch}",
                                       tag="proj", bufs=3)
                        mm(rot, prot, kraw)
                        rope(kt_sb[:, ch * 512:(ch + 1) * 512], kraw, rot,
                             ck[:, ch * 512:(ch + 1) * 512],
                             sk[:, ch * 512:(ch + 1) * 512], 512)
                    k2.append(kt_sb)

                # --- V projection (token-major, 65-col per head with ones col) ---
                vg = [pa.tile([128, 4, 65], BF16, name=f"v_{g}_{kt}", tag=f"v{kt}", bufs=2)
                      for kt in range(NT)]
                for kt2 in range(4):
                    psv = psa.tile([128, 512], F32, name=f"vps_{g}_{kt2}", tag="proj", bufs=3)
                    for half in (0, 1):
                        kt = 2 * kt2 + half
                        for k in range(NT):
                            mm(psv[:, half * 256:(half + 1) * 256],
                               xw_b[:, k, kt * 128:(kt + 1) * 128],
                               wv_sb[:, k, g * 256:(g + 1) * 256],
                               start=(k == 0), stop=(k == NT - 1))
                    for half in (0, 1):
                        kt = 2 * kt2 + half
                        nc.vector.tensor_copy(
                            vg[kt][:, :, 64:65],
                            onesb[:, 0:1].unsqueeze(1).broadcast_to([128, 4, 1]))
                        nc.vector.tensor_copy(
                            vg[kt][:, :, 0:64],
                            psv[:, half * 256:(half + 1) * 256].rearrange(
                                "p (h c) -> p h c", c=64))

                # --- attention per head pair ---
                for p2 in range(2):
                    vac = [psa.tile([65, 512], F32, name=f"vac_{g}_{p2}_{jj}",
                                    tag="vac", bufs=3) for jj in range(2)]
                    for kt in range(NT):
                        for jj in range(2):
                            j = 2 * p2 + jj
                            sc = psa.tile([128, 512], F32, name=f"sc_{g}_{p2}_{kt}_{jj}",
                                          tag="sc", bufs=2)
                            mm(sc, k2[p2][64 * jj:64 * (jj + 1), kt * 128:(kt + 1) * 128],
                               q2[p2][64 * jj:64 * (jj + 1), :],
                               start=True, stop=True, tile_position=(64 * jj, 0))
                            at = pa.tile([128, 512], BF16, name=f"at_{g}_{p2}_{kt}_{jj}",
                                         tag="at", bufs=3)
                            nc.scalar.activation(at, sc, AF.Exp, scale=0.125)
                            mm(vac[jj], vg[kt][:, j, :], at,
                               start=(kt == 0), stop=(kt == NT - 1))
                    # normalize pair -> attention out tile i (heads 2i, 2i+1)
                    rr = pa.tile([1, 1024], BF16, name=f"rr_{g}_{p2}", tag="rr", bufs=2)
                    nc.scalar.copy(rr[0:1, 0:512], vac[0][64:65, :])
                    nc.scalar.copy(rr[0:1, 512:1024], vac[1][64:65, :])
                    bcs = psa.tile([128, 512], F32, name=f"bcs_{g}_{p2}", tag="vac", bufs=3)
                    mm(bcs[0:64, :], ones64, rr[0:1, 0:512])
                    mm(bcs[64:128, :], ones64, rr[0:1, 512:1024], tile_position=(0, 64))
                    rbc = pa.tile([128, 512], F32, name=f"rbc_{g}_{p2}", tag="rbc", bufs=2)
                    nc.vector.reciprocal(rbc, bcs)
                    i = 2 * g + p2
                    nc.vector.tensor_mul(ao[i][0:64, :], vac[0][0:64, :], rbc[0:64, :])
                    nc.vector.tensor_mul(ao[i][64:128, :], vac[1][0:64, :], rbc[64:128, :])

            # ---------------- Stage B: out-proj + RMSNorm1 -> h1 ----------------
            ssp = psa.tile([1, 512], F32, name="ssp", tag="sc", bufs=2)
            for m in range(NT):
                yp = psa.tile([128, LQ], F32, name=f"yps_{m}", tag="proj", bufs=3)
                for k in range(NT):
                    mm(yp, wo_sb[:, k, m * 128:(m + 1) * 128], ao[k],
                       start=(k == 0), stop=(k == NT - 1))
                nc.vector.tensor_add(s_sb[m], xq_f[:, m, :], yp)
                sqt = pa.tile([128, LQ], BF16, name=f"sq1_{m}", tag="sqv", bufs=2)
                nc.vector.tensor_mul(sqt, s_sb[m], s_sb[m])
                mm(ssp, onesK, sqt, start=(m == 0), stop=(m == NT - 1))
            row = pa.tile([1, 512], F32, name="row1", tag="row", bufs=2)
            nc.scalar.activation(row, ssp, AF.Sqrt, scale=1.0 / WIDTH, bias=eps1)
            nc.vector.reciprocal(row, row)
            rowb = pa.tile([1, 512], BF16, name="rowb1", tag="rowb", bufs=2)
            nc.scalar.copy(rowb, row)
            bcn = psa.tile([128, 512], F32, name="bcn", tag="vac", bufs=3)
            mm(bcn, ones1, rowb)
            for m in range(NT):
                nc.vector.scalar_tensor_tensor(h1f[m], s_sb[m], g1[:, m:m + 1], bcn,
                                               op0=OP.mult, op1=OP.mult)
                nc.scalar.copy(h1b[m], h1f[m])

        # ---------------- Stage C: SwiGLU FFN + RMSNorm2 ----------------
        with tc.tile_pool(name="pcn", bufs=1) as pcn, \
             tc.tile_pool(name="wc", bufs=1) as wc, \
             tc.tile_pool(name="psc", bufs=1, space="PSUM") as psc:
            xr_b = pcn.tile([128, NT, REST], BF16, name="xr_b")
            nc.sync.dma_start(xr_b, A["xr_b"])
            F_t = [pcn.tile([128, 1024], BF16, name=f"F_{h}", tag=f"F{h}") for h in range(NH)]
            s2 = [pcn.tile([128, 1024], F32, name=f"s2_{m}", tag=f"s2_{m}") for m in range(NT)]

            def tin(s, k, lo=0, hi=512):
                # token sub-tile s (of 4): bf16 matmul operand, feature tile k
                if s == 0:
                    return h1b[k][:, lo:hi]
                return xr_b[:, k, (s - 1) * 512 + lo:(s - 1) * 512 + hi]

            def resid(s, m):
                # fp32-ish residual base for sub-tile s, feature tile m
                if s == 0:
                    return h1f[m]
                return xr_b[:, m, (s - 1) * 512:s * 512]

            for c in range(2):
                for hm in range(NH):
                    wgu = wc.tile([128, 2, NT, 128], BF16, name=f"wgu_{c}_{hm}",
                                  tag="wgu", bufs=3)
                    nc.sync.dma_start(wgu, wgu_r[hm])
                    for t in (0, 1):
                        s = 2 * c + t
                        gp = psc.tile([128, 512], F32, name=f"gp_{c}_{hm}_{t}",
                                      tag="gu", bufs=4)
                        for k in range(NT):
                            mm(gp, wgu[:, 0, k, :], tin(s, k),
                               start=(k == 0), stop=(k == NT - 1))
                        up = psc.tile([128, 512], F32, name=f"up_{c}_{hm}_{t}",
                                      tag="gu", bufs=4)
                        for k in range(NT):
                            mm(up, wgu[:, 1, k, :], tin(s, k),
                               start=(k == 0), stop=(k == NT - 1))
                        sg = pcn.tile([128, 512], BF16, name=f"sg_{c}_{hm}_{t}",
                                      tag="sg", bufs=2)
                        nc.scalar.activation(sg, gp, AF.Silu)
                        nc.vector.tensor_mul(F_t[hm][:, t * 512:(t + 1) * 512], sg, up)

                ssp2 = [psc.tile([1, 512], F32, name=f"ssp2_{c}_{t}", tag="st", bufs=2)
                        for t in (0, 1)]
                for m in range(NT):
                    wdb = wc.tile([128, NH, 128], BF16, name=f"wdb_{c}_{m}", tag="wd", bufs=2)
                    nc.sync.dma_start(wdb, wd_r[m])
                    for t in (0, 1):
                        s = 2 * c + t
                        dp = psc.tile([128, 512], F32, name=f"dp_{c}_{m}_{t}", tag="d", bufs=2)
                        for hk in range(NH):
                            mm(dp, wdb[:, hk, :], F_t[hk][:, t * 512:(t + 1) * 512],
                               start=(hk == 0), stop=(hk == NH - 1))
                        s2t = s2[m][:, t * 512:(t + 1) * 512]
                        nc.vector.tensor_add(s2t, resid(s, m), dp)
                        sqt = pcn.tile([128, 512], BF16, name=f"sq2_{c}_{m}_{t}",
                                       tag="sg", bufs=2)
                        nc.vector.tensor_mul(sqt, s2t, s2t)
                        mm(ssp2[t], onesK, sqt, start=(m == 0), stop=(m == NT - 1))
                for t in (0, 1):
                    s = 2 * c + t
                    row2 = pcn.tile([1, 512], F32, name=f"row2_{c}_{t}", tag="row2", bufs=2)
                    nc.scalar.activation(row2, ssp2[t], AF.Sqrt, scale=1.0 / WIDTH, bias=eps1)
                    nc.vector.reciprocal(row2, row2)
                    rowb2 = pcn.tile([1, 512], BF16, name=f"rowb2_{c}_{t}", tag="rowb2", bufs=2)
                    nc.scalar.copy(rowb2, row2)
                    bc2 = psc.tile([128, 512], F32, name=f"bc2_{c}_{t}", tag="d", bufs=2)
                    mm(bc2, ones1, rowb2)
                    for m in range(NT):
                        ot = pcn.tile([128, 512], F32, name=f"ot_{c}_{t}_{m}", tag="ot", bufs=3)
                        nc.vector.scalar_tensor_tensor(
                            ot, s2[m][:, t * 512:(t + 1) * 512], g2[:, m:m + 1], bc2,
                            op0=OP.mult, op1=OP.mult)
                        nc.sync.dma_start(out_r[m][:, s * 512:(s + 1) * 512], ot)


_INPUT_SPECS = [
    ("xw_b", [128, NT, LWIN], BF16),
    ("xq_b", [128, NT, LQ], BF16),
    ("xq_f", [128, NT, LQ], F32),
    ("xr_b", [128, NT, REST], BF16),
    ("wq_p", [128, NT, NT, 128], BF16),
    ("wk_p", [128, NT, NT, 128], BF16),
    ("wv_p", [128, NT, WIDTH], BF16),
    ("wo_p", [128, NT, WIDTH], BF16),
    ("wgu_p", [128, NH, 2, NT, 128], BF16),
    ("wd_p", [128, NT, NH, 128], BF16),
    ("rope_p", [128, 128], BF16),
    ("ones_b", [128, 128], BF16),
    ("cos_q", [128, LQ], F32),
    ("sin_q", [128, LQ], F32),
    ("cos_k", [128, LWIN], F32),
    ("sin_k", [128, LWIN], F32),
    ("g1", [128, NT], F32),
    ("g2", [128, NT], F32),
]


def build_program(reps=1):
    nc = bacc.Bacc("TRN2", target_bir_lowering=False, debug=False, num_devices=N_CORES)
    A = {name: nc.dram_tensor(name, shape, dt, kind="ExternalInput").ap()
         for name, shape, dt in _INPUT_SPECS}
    out_ap = nc.dram_tensor("out_t", [128, NT, TOUT], F32, kind="ExternalOutput").ap()
    with tile.TileContext(nc) as tc:
        for _ in range(reps):
            _emit(tc, A, out_ap)
    nc.compile()
    return nc


def _tileize(a, inner):
    # [K*128, M] -> [128, K, M] with partition p = row % 128 within each k block
    K = a.shape[0] // 128
    return np.ascontiguousarray(a.reshape(K, 128, *a.shape[1:]).transpose(1, 0, 2))


def make_in_maps(x, w_qkv, w_out, g_norm1, g_norm2, w_gate, w_up, w_down):
    f32 = np.float32
    x = np.asarray(x, f32)
    w_qkv = np.asarray(w_qkv, f32)
    # head-contiguous de-interleave: within head h, even dims first then odd:
    # new row h*64+j -> old h*64+2j ; new row h*64+32+j -> old h*64+2j+1
    perm = np.empty(WIDTH, np.int64)
    for h in range(HEADS):
        j = np.arange(32)
        perm[h * 64 + j] = h * 64 + 2 * j
        perm[h * 64 + 32 + j] = h * 64 + 2 * j + 1
    wq = w_qkv[0:WIDTH][perm]
    wk = w_qkv[WIDTH:2 * WIDTH][perm]
    wv = w_qkv[2 * WIDTH:3 * WIDTH]

    def pack_qk(w):
        # [128, k, m, 128]: [p, k, m, j] = w.T[k*128+p, m*128+j]
        a = _tileize(w.T.astype(BF), None)          # [128, 8, 1024]
        return np.ascontiguousarray(a.reshape(128, NT, NT, 128))

    wgu = np.stack([np.asarray(w_gate, f32).T, np.asarray(w_up, f32).T], axis=1)
    # wgu: [1024, 2, 4096] -> [128, hm, gu, k, 128]
    a = wgu.reshape(NT, 128, 2, NH, 128).transpose(1, 3, 2, 0, 4)
    wgu_p = np.ascontiguousarray(a.astype(BF))
    # wd: [4096, 1024] (w_down.T) -> [128, m, hk, 128]
    a = np.asarray(w_down, f32).T.reshape(NH, 128, NT, 128).transpose(1, 2, 0, 3)
    wd_p = np.ascontiguousarray(a.astype(BF))

    # RoPE permutation matrix P as lhsT: out even j <- -odd j ; out odd j <- +even j
    P = np.zeros((128, 128), f32)
    for hh in (0, 64):
        j = np.arange(32)
        P[hh + 32 + j, hh + j] = -1.0
        P[hh + j, hh + 32 + j] = 1.0

    inv_freq = (1.0 / (ROPE_BASE ** (np.arange(0, HDIM, 2, dtype=np.float64) / HDIM)))

    def tab(pos):
        fr = np.outer(inv_freq, pos.astype(np.float64))  # [32, T]
        return (np.tile(np.cos(fr), (4, 1)).astype(f32),
                np.tile(np.sin(fr), (4, 1)).astype(f32))

    cos_k, sin_k = tab(np.arange(LWIN))
    common = {
        "wq_p": pack_qk(wq),
        "wk_p": pack_qk(wk),
        "wv_p": _tileize(wv.T.astype(BF), None),
        "wo_p": _tileize(np.asarray(w_out, f32).T.astype(BF), None),
        "wgu_p": wgu_p,
        "wd_p": wd_p,
        "rope_p": P.astype(BF),
        "ones_b": np.ones((128, 128), BF),
        "cos_k": cos_k,
        "sin_k": sin_k,
        "g1": np.ascontiguousarray(np.asarray(g_norm1, f32).reshape(NT, 128).T),
        "g2": np.ascontiguousarray(np.asarray(g_norm2, f32).reshape(NT, 128).T),
    }
    in_maps = []
    for c in range(N_CORES):
        b, qh = c // 2, c % 2
        cos_q, sin_q = tab(np.arange(qh * LQ, (qh + 1) * LQ))
        m = dict(common)
        xq = x[b, qh * LQ:(qh + 1) * LQ].T               # [1024, 512]
        m["xw_b"] = _tileize(x[b, :LWIN].T.astype(BF), None)
        m["xq_b"] = _tileize(xq.astype(BF), None)
        m["xq_f"] = _tileize(xq, None)
        m["xr_b"] = _tileize(
            x[b, LWIN + qh * REST:LWIN + (qh + 1) * REST].T.astype(BF), None)
        m["cos_q"] = cos_q
        m["sin_q"] = sin_q
        in_maps.append(m)
    return in_maps


def assemble_output(results):
    out = np.empty((4, 4096, WIDTH), np.float32)
    for c in range(N_CORES):
        b, qh = c // 2, c % 2
        o = results[c]["out_t"].reshape(128, NT, TOUT)
        ot = o.transpose(2, 1, 0).reshape(TOUT, WIDTH)   # [token, width]
        out[b, qh * LQ:(qh + 1) * LQ] = ot[:LQ]
        out[b, LWIN + qh * REST:LWIN + (qh + 1) * REST] = ot[LQ:]
    return out


_CACHE = {}


def kernel(x, w_qkv, w_out, g_norm1, g_norm2, w_gate, w_up, w_down):
    if "nc" not in _CACHE:
        _CACHE["nc"] = build_program()
    nc = _CACHE["nc"]
    in_maps = make_in_maps(x, w_qkv, w_out, g_norm1, g_norm2, w_gate, w_up, w_down)
    res = run_bass_kernel_spmd(nc, in_maps, list(range(N_CORES))).results
    return assemble_output(res)


# revision 17
# speedup vs baseline: 1.3792x; 1.0053x over previous
"""Trainium2 Bass kernel for ContextualAttentionBlock.

Sharding: 8 cores, core c -> (batch b = c//2, query-half qh = c%2).
Each core computes, for its batch's 1024-token attention window:
  K/V projections for all 1024 tokens, Q for its 512 queries, RoPE,
  attention, out-proj, residual+RMSNorm1 -> h1 (512 tokens),
then SwiGLU FFN + residual + RMSNorm2 for 2048 tokens
  (512 attention-part tokens + 1536 "rest" tokens that skip attention).

All matmul operands are bf16 (fp32 PSUM accumulation); residual/norm
statistics stay fp32.  Weights are host-packed into per-DMA contiguous
blobs so each weight load is one large descriptor-friendly transfer.
RoPE is computed as  x*cos + (P@x)*sin  where P is a constant +-1
permutation matrix applied on the tensor engine, so the vector engine
only does 3 full-width ops per tile.
No collectives; the host shards inputs and reassembles the output.
"""

import numpy as np
import ml_dtypes

import concourse.bass as bass
import concourse.tile as tile
from concourse import bacc, mybir
from concourse.bass_utils import run_bass_kernel_spmd

F32 = mybir.dt.float32
BF16 = mybir.dt.bfloat16
AF = mybir.ActivationFunctionType
OP = mybir.AluOpType

WIDTH = 1024
NT = 8              # width tiles of 128
HEADS = 16
HDIM = 64
LWIN = 1024         # attention window
LQ = 512            # queries per core
HID = 4096
NH = 32             # hidden tiles of 128
REST = 1536         # rest tokens per core
TOUT = LQ + REST    # 2048 tokens through the FFN per core
NSUB = 4            # 4 token sub-tiles of 512: [h1, xr0, xr1, xr2]
EPS = 1e-6
ROPE_BASE = 10000.0
N_CORES = 8
BF = ml_dtypes.bfloat16


def _emit(tc, A, out_ap):
    nc = tc.nc
    mm = nc.tensor.matmul
    wgu_r = A["wgu_p"].rearrange("p h g k j -> h p g k j")
    wd_r = A["wd_p"].rearrange("p m h j -> m p h j")
    out_r = out_ap.rearrange("p m t -> m p t")

    with tc.tile_pool(name="pc", bufs=1) as pc:
        onesb = pc.tile([128, 128], BF16, name="onesb")
        nc.sync.dma_start(onesb, A["ones_b"])
        onesK = onesb[:, 0:1]       # [128,1]
        ones1 = onesb[0:1, :]       # [1,128]
        ones64 = onesb[0:1, 0:64]   # [1,64]
        eps1 = pc.tile([1, 1], F32, name="eps1")
        nc.vector.memset(eps1, EPS)
        g1 = pc.tile([128, NT], F32, name="g1")
        nc.sync.dma_start(g1, A["g1"])
        g2 = pc.tile([128, NT], F32, name="g2")
        nc.sync.dma_start(g2, A["g2"])
        h1f = [pc.tile([128, LQ], F32, name=f"h1f_{m}", tag=f"h1f{m}") for m in range(NT)]
        h1b = [pc.tile([128, LQ], BF16, name=f"h1b_{m}", tag=f"h1b{m}") for m in range(NT)]

        # ---------------- Stage A: attention ----------------
        with tc.tile_pool(name="pa", bufs=1) as pa, \
             tc.tile_pool(name="wa", bufs=1) as wa, \
             tc.tile_pool(name="psa", bufs=1, space="PSUM") as psa:
            wq_r = A["wq_p"].rearrange("p k (gg ss) j -> gg p k ss j", ss=2)
            wk_r = A["wk_p"].rearrange("p k (gg ss) j -> gg p k ss j", ss=2)
            # DMA issue order tracks the tensor-engine critical path:
            # V-proj inputs first, then Q, then K tables, then late stage-B
            # tensors (emitted after the g=0 block below).
            xw_b = pa.tile([128, NT, LWIN], BF16, name="xw_b")
            nc.sync.dma_start(xw_b, A["xw_b"])
            wv_sb = wa.tile([128, NT, WIDTH], BF16, name="wv_sb")
            nc.sync.dma_start(wv_sb, A["wv_p"])
            xq_b = pa.tile([128, NT, LQ], BF16, name="xq_b")
            nc.sync.dma_start(xq_b, A["xq_b"])
            prot = wa.tile([128, 128], BF16, name="prot")
            nc.sync.dma_start(prot, A["rope_p"])
            cq = pa.tile([128, LQ], F32, name="cq")
            nc.sync.dma_start(cq, A["cos_q"])
            sq = pa.tile([128, LQ], F32, name="sq")
            nc.sync.dma_start(sq, A["sin_q"])

            ao = [pa.tile([128, LQ], BF16, name=f"ao_{i}", tag=f"ao{i}") for i in range(NT)]
            s_sb = [pa.tile([128, LQ], F32, name=f"s_{m}", tag=f"s{m}") for m in range(NT)]

            # --- V projection for all 16 heads (token-major, 65-col per head
            # with a ones column for the softmax denominator) ---
            vg = [pa.tile([128, HEADS, 65], BF16, name=f"v_{kt}", tag=f"v{kt}")
                  for kt in range(NT)]
            for kt in range(NT):
                for vch in range(2):
                    psv = psa.tile([128, 512], F32, name=f"vps_{kt}_{vch}",
                                   tag="proj", bufs=4)
                    for k in range(NT):
                        mm(psv, xw_b[:, k, kt * 128:(kt + 1) * 128],
                           wv_sb[:, k, vch * 512:(vch + 1) * 512],
                           start=(k == 0), stop=(k == NT - 1))
                    nc.vector.tensor_copy(
                        vg[kt][:, vch * 8:(vch + 1) * 8, 0:64],
                        psv.rearrange("p (h c) -> p h c", c=64))
                nc.vector.tensor_copy(
                    vg[kt][:, :, 64:65],
                    onesb[:, 0:1].unsqueeze(1).broadcast_to([128, HEADS, 1]))

            ck = pa.tile([128, LWIN], F32, name="ck")
            nc.sync.dma_start(ck, A["cos_k"])
            sk = pa.tile([128, LWIN], F32, name="sk")
            nc.sync.dma_start(sk, A["sin_k"])
            # stage-B tensors ride the Activation HWDGE queue so they don't
            # delay the SP-queue weight stream
            wo_sb = wa.tile([128, NT, WIDTH], BF16, name="wo_sb")
            nc.scalar.dma_start(wo_sb, A["wo_p"])
            xq_f = pa.tile([128, NT, LQ], F32, name="xq_f")
            nc.scalar.dma_start(xq_f, A["xq_f"])

            def rope(dest, raw_sb, rot_ps, cos_t, sin_t, w):
                # dest = raw*cos + (P@raw)*sin, all [128, w] full-width ops
                t1 = pa.tile([128, w], F32, name="rt1", tag="rt1", bufs=2)
                nc.vector.tensor_mul(t1, raw_sb, cos_t)
                t2 = pa.tile([128, w], F32, name="rt2", tag="rt2", bufs=2)
                nc.vector.tensor_mul(t2, rot_ps, sin_t)
                nc.vector.tensor_add(dest, t1, t2)

            for g in range(4):
                # --- Q projection + RoPE (tiles 2g, 2g+1; heads 4g..4g+3) ---
                wqg = wa.tile([128, NT, 2, 128], BF16, name=f"wqg_{g}", tag="wqg", bufs=2)
                nc.sync.dma_start(wqg, wq_r[g])
                wkg = wa.tile([128, NT, 2, 128], BF16, name=f"wkg_{g}", tag="wkg", bufs=2)
                nc.sync.dma_start(wkg, wk_r[g])
                q2 = []
                for side in (0, 1):
                    ps = psa.tile([128, LQ], F32, name=f"qps_{g}_{side}", tag="proj", bufs=4)
                    for k in range(NT):
                        mm(ps, wqg[:, k, side, :], xq_b[:, k, :],
                           start=(k == 0), stop=(k == NT - 1))
                    qraw = pa.tile([128, LQ], BF16, name=f"qraw_{g}_{side}", tag="raw", bufs=2)
                    nc.scalar.copy(qraw, ps)
                    rot = psa.tile([128, LQ], F32, name=f"qrot_{g}_{side}", tag="proj", bufs=4)
                    mm(rot, prot, qraw)
                    qt = pa.tile([128, LQ], BF16, name=f"q_{g}_{side}",
                                 tag=("q0" if side == 0 else "q1"), bufs=2)
                    rope(qt, qraw, rot, cq, sq, LQ)
                    q2.append(qt)

                # --- K projection + RoPE ---
                k2 = []
                for side in (0, 1):
                    kt_sb = pa.tile([128, LWIN], BF16, name=f"k_{g}_{side}",
                                    tag=("k0" if side == 0 else "k1"), bufs=2)
                    for ch in range(2):
                        ps = psa.tile([128, 512], F32, name=f"kps_{g}_{side}_{ch}",
                                      tag="proj", bufs=4)
                        for k in range(NT):
                            mm(ps, wkg[:, k, side, :], xw_b[:, k, ch * 512:(ch + 1) * 512],
                               start=(k == 0), stop=(k == NT - 1))
                        kraw = pa.tile([128, 512], BF16, name=f"kraw_{g}_{side}_{ch}",
                                       tag="raw", bufs=2)
                        nc.scalar.copy(kraw, ps)
                        rot = psa.tile([128, 512], F32, name=f"krot_{g}_{side}_{ch}",
                                       tag="proj", bufs=4)
                        mm(rot, prot, kraw)
                        rope(kt_sb[:, ch * 512:(ch + 1) * 512], kraw, rot,
                             ck[:, ch * 512:(ch + 1) * 512],
                             sk[:, ch * 512:(ch + 1) * 512], 512)
                    k2.append(kt_sb)

                # --- attention per head pair (scores pipelined one kt ahead
                # of the attn@V accumulation so the Exp latency is hidden) ---
                for p2 in range(2):
                    vac = [psa.tile([65, 512], F32, name=f"vac_{g}_{p2}_{jj}",
                                    tag="vac", bufs=3) for jj in range(2)]
                    at_t = {}

                    def emit_sc(kt, jj, g=g, p2=p2, k2=k2, q2=q2, at_t=at_t):
                        sc = psa.tile([128, 512], F32, name=f"sc_{g}_{p2}_{kt}_{jj}",
                                      tag="proj", bufs=4)
                        mm(sc, k2[p2][64 * jj:64 * (jj + 1), kt * 128:(kt + 1) * 128],
                           q2[p2][64 * jj:64 * (jj + 1), :],
                           start=True, stop=True, tile_position=(64 * jj, 0))
                        at = pa.tile([128, 512], BF16, name=f"at_{g}_{p2}_{kt}_{jj}",
                                     tag="at", bufs=4)
                        nc.scalar.activation(at, sc, AF.Exp, scale=0.125)
                        at_t[(kt, jj)] = at

                    emit_sc(0, 0)
                    emit_sc(0, 1)
                    for kt in range(NT):
                        if kt + 1 < NT:
                            emit_sc(kt + 1, 0)
                            emit_sc(kt + 1, 1)
                        for jj in range(2):
                            mm(vac[jj], vg[kt][:, 4 * g + 2 * p2 + jj, :],
                               at_t.pop((kt, jj)),
                               start=(kt == 0), stop=(kt == NT - 1))
                    # normalize pair -> attention out tile i (heads 2i, 2i+1)
                    rr = pa.tile([1, 1024], BF16, name=f"rr_{g}_{p2}", tag="rr", bufs=2)
                    nc.scalar.copy(rr[0:1, 0:512], vac[0][64:65, :])
                    nc.scalar.copy(rr[0:1, 512:1024], vac[1][64:65, :])
                    bcs = psa.tile([128, 512], F32, name=f"bcs_{g}_{p2}", tag="vac", bufs=3)
                    mm(bcs[0:64, :], ones64, rr[0:1, 0:512])
                    mm(bcs[64:128, :], ones64, rr[0:1, 512:1024], tile_position=(0, 64))
                    rbc = pa.tile([128, 512], F32, name=f"rbc_{g}_{p2}", tag="rbc", bufs=2)
                    nc.vector.reciprocal(rbc, bcs)
                    i = 2 * g + p2
                    nc.vector.tensor_mul(ao[i][0:64, :], vac[0][0:64, :], rbc[0:64, :])
                    nc.vector.tensor_mul(ao[i][64:128, :], vac[1][0:64, :], rbc[64:128, :])

            # ---------------- Stage B: out-proj + RMSNorm1 -> h1 ----------------
            ssp = psa.tile([1, 512], F32, name="ssp", tag="st1", bufs=1)
            for m in range(NT):
                yp = psa.tile([128, LQ], F32, name=f"yps_{m}", tag="proj", bufs=4)
                for k in range(NT):
                    mm(yp, wo_sb[:, k, m * 128:(m + 1) * 128], ao[k],
                       start=(k == 0), stop=(k == NT - 1))
                nc.vector.tensor_add(s_sb[m], xq_f[:, m, :], yp)
                sqt = pa.tile([128, LQ], BF16, name=f"sq1_{m}", tag="sqv", bufs=2)
                nc.vector.tensor_mul(sqt, s_sb[m], s_sb[m])
                mm(ssp, onesK, sqt, start=(m == 0), stop=(m == NT - 1))
            row = pa.tile([1, 512], F32, name="row1", tag="row", bufs=2)
            nc.scalar.activation(row, ssp, AF.Sqrt, scale=1.0 / WIDTH, bias=eps1)
            nc.vector.reciprocal(row, row)
            rowb = pa.tile([1, 512], BF16, name="rowb1", tag="rowb", bufs=2)
            nc.scalar.copy(rowb, row)
            bcn = psa.tile([128, 512], F32, name="bcn", tag="vac", bufs=3)
            mm(bcn, ones1, rowb)
            for m in range(NT):
                nc.vector.scalar_tensor_tensor(h1f[m], s_sb[m], g1[:, m:m + 1], bcn,
                                               op0=OP.mult, op1=OP.mult)
                nc.scalar.copy(h1b[m], h1f[m])

        # ---------------- Stage C: SwiGLU FFN + RMSNorm2 ----------------
        with tc.tile_pool(name="pcn", bufs=1) as pcn, \
             tc.tile_pool(name="wc", bufs=1) as wc, \
             tc.tile_pool(name="psc", bufs=1, space="PSUM") as psc:
            xr_b = pcn.tile([128, NT, REST], BF16, name="xr_b")
            nc.sync.dma_start(xr_b, A["xr_b"])
            F_t = [pcn.tile([128, 1024], BF16, name=f"F_{h}", tag=f"F{h}") for h in range(NH)]
            s2 = [pcn.tile([128, 1024], F32, name=f"s2_{m}", tag=f"s2_{m}") for m in range(NT)]

            def tin(s, k, lo=0, hi=512):
                # token sub-tile s (of 4): bf16 matmul operand, feature tile k
                if s == 0:
                    return h1b[k][:, lo:hi]
                return xr_b[:, k, (s - 1) * 512 + lo:(s - 1) * 512 + hi]

            def resid(s, m):
                # fp32-ish residual base for sub-tile s, feature tile m
                if s == 0:
                    return h1f[m]
                return xr_b[:, m, (s - 1) * 512:s * 512]

            for c in range(2):
                for hm in range(NH):
                    wgu = wc.tile([128, 2, NT, 128], BF16, name=f"wgu_{c}_{hm}",
                                  tag="wgu", bufs=3)
                    nc.sync.dma_start(wgu, wgu_r[hm])
                    for t in (0, 1):
                        s = 2 * c + t
                        gp = psc.tile([128, 512], F32, name=f"gp_{c}_{hm}_{t}",
                                      tag="gu", bufs=4)
                        for k in range(NT):
                            mm(gp, wgu[:, 0, k, :], tin(s, k),
                               start=(k == 0), stop=(k == NT - 1))
                        up = psc.tile([128, 512], F32, name=f"up_{c}_{hm}_{t}",
                                      tag="gu", bufs=4)
                        for k in range(NT):
                            mm(up, wgu[:, 1, k, :], tin(s, k),
                               start=(k == 0), stop=(k == NT - 1))
                        sg = pcn.tile([128, 512], BF16, name=f"sg_{c}_{hm}_{t}",
                                      tag="sg", bufs=2)
                        nc.scalar.activation(sg, gp, AF.Silu)
                        nc.vector.tensor_mul(F_t[hm][:, t * 512:(t + 1) * 512], sg, up)

                ssp2 = [psc.tile([1, 512], F32, name=f"ssp2_{c}_{t}", tag="st", bufs=2)
                        for t in (0, 1)]
                for m in range(NT):
                    wdb = wc.tile([128, NH, 128], BF16, name=f"wdb_{c}_{m}", tag="wd", bufs=2)
                    nc.sync.dma_start(wdb, wd_r[m])
                    for t in (0, 1):
                        s = 2 * c + t
                        dp = psc.tile([128, 512], F32, name=f"dp_{c}_{m}_{t}", tag="d", bufs=2)
                        for hk in range(NH):
                            mm(dp, wdb[:, hk, :], F_t[hk][:, t * 512:(t + 1) * 512],
                               start=(hk == 0), stop=(hk == NH - 1))
                        s2t = s2[m][:, t * 512:(t + 1) * 512]
                        nc.vector.tensor_add(s2t, resid(s, m), dp)
                        sqt = pcn.tile([128, 512], BF16, name=f"sq2_{c}_{m}_{t}",
                                       tag="sg", bufs=2)
                        nc.vector.tensor_mul(sqt, s2t, s2t)
                        mm(ssp2[t], onesK, sqt, start=(m == 0), stop=(m == NT - 1))
                for t in (0, 1):
                    s = 2 * c + t
                    row2 = pcn.tile([1, 512], F32, name=f"row2_{c}_{t}", tag="row2", bufs=2)
                    nc.scalar.activation(row2, ssp2[t], AF.Sqrt, scale=1.0 / WIDTH, bias=eps1)
                    nc.vector.reciprocal(row2, row2)
                    rowb2 = pcn.tile([1, 512], BF16, name=f"rowb2_{c}_{t}", tag="rowb2", bufs=2)
                    nc.scalar.copy(rowb2, row2)
                    bc2 = psc.tile([128, 512], F32, name=f"bc2_{c}_{t}", tag="d", bufs=2)
                    mm(bc2, ones1, rowb2)
                    for m in range(NT):
                        ot = pcn.tile([128, 512], F32, name=f"ot_{c}_{t}_{m}", tag="ot", bufs=3)
                        nc.vector.scalar_tensor_tensor(
                            ot, s2[m][:, t * 512:(t + 1) * 512], g2[:, m:m + 1], bc2,
                            op0=OP.mult, op1=OP.mult)
                        nc.sync.dma_start(out_r[m][:, s * 512:(s + 1) * 512], ot)


_INPUT_SPECS = [
    ("xw_b", [128, NT, LWIN], BF16),
    ("xq_b", [128, NT, LQ], BF16),
    ("xq_f", [128, NT, LQ], F32),
    ("xr_b", [128, NT, REST], BF16),
    ("wq_p", [128, NT, NT, 128], BF16),
    ("wk_p", [128, NT, NT, 128], BF16),
    ("wv_p", [128, NT, WIDTH], BF16),
    ("wo_p", [128, NT, WIDTH], BF16),
    ("wgu_p", [128, NH, 2, NT, 128], BF16),
    ("wd_p", [128, NT, NH, 128], BF16),
    ("rope_p", [128, 128], BF16),
    ("ones_b", [128, 128], BF16),
    ("cos_q", [128, LQ], F32),
    ("sin_q", [128, LQ], F32),
    ("cos_k", [128, LWIN], F32),
    ("sin_k", [128, LWIN], F32),
    ("g1", [128, NT], F32),
    ("g2", [128, NT], F32),
]


def build_program(reps=1):
    nc = bacc.Bacc("TRN2", target_bir_lowering=False, debug=False, num_devices=N_CORES)
    A = {name: nc.dram_tensor(name, shape, dt, kind="ExternalInput").ap()
         for name, shape, dt in _INPUT_SPECS}
    out_ap = nc.dram_tensor("out_t", [128, NT, TOUT], F32, kind="ExternalOutput").ap()
    with tile.TileContext(nc) as tc:
        for _ in range(reps):
            _emit(tc, A, out_ap)
    nc.compile()
    return nc


def _tileize(a, inner):
    # [K*128, M] -> [128, K, M] with partition p = row % 128 within each k block
    K = a.shape[0] // 128
    return np.ascontiguousarray(a.reshape(K, 128, *a.shape[1:]).transpose(1, 0, 2))


def make_in_maps(x, w_qkv, w_out, g_norm1, g_norm2, w_gate, w_up, w_down):
    f32 = np.float32
    x = np.asarray(x, f32)
    w_qkv = np.asarray(w_qkv, f32)
    # head-contiguous de-interleave: within head h, even dims first then odd:
    # new row h*64+j -> old h*64+2j ; new row h*64+32+j -> old h*64+2j+1
    perm = np.empty(WIDTH, np.int64)
    for h in range(HEADS):
        j = np.arange(32)
        perm[h * 64 + j] = h * 64 + 2 * j
        perm[h * 64 + 32 + j] = h * 64 + 2 * j + 1
    wq = w_qkv[0:WIDTH][perm]
    wk = w_qkv[WIDTH:2 * WIDTH][perm]
    wv = w_qkv[2 * WIDTH:3 * WIDTH]

    def pack_qk(w):
        # [128, k, m, 128]: [p, k, m, j] = w.T[k*128+p, m*128+j]
        a = _tileize(w.T.astype(BF), None)          # [128, 8, 1024]
        return np.ascontiguousarray(a.reshape(128, NT, NT, 128))

    wgu = np.stack([np.asarray(w_gate, f32).T, np.asarray(w_up, f32).T], axis=1)
    # wgu: [1024, 2, 4096] -> [128, hm, gu, k, 128]
    a = wgu.reshape(NT, 128, 2, NH, 128).transpose(1, 3, 2, 0, 4)
    wgu_p = np.ascontiguousarray(a.astype(BF))
    # wd: [4096, 1024] (w_down.T) -> [128, m, hk, 128]
    a = np.asarray(w_down, f32).T.reshape(NH, 128, NT, 128).transpose(1, 2, 0, 3)
    wd_p = np.ascontiguousarray(a.astype(BF))

    # RoPE permutation matrix P as lhsT: out even j <- -odd j ; out odd j <- +even j
    P = np.zeros((128, 128), f32)
    for hh in (0, 64):
        j = np.arange(32)
        P[hh + 32 + j, hh + j] = -1.0
        P[hh + j, hh + 32 + j] = 1.0

    inv_freq = (1.0 / (ROPE_BASE ** (np.arange(0, HDIM, 2, dtype=np.float64) / HDIM)))

    def tab(pos):
        fr = np.outer(inv_freq, pos.astype(np.float64))  # [32, T]
        return (np.tile(np.cos(fr), (4, 1)).astype(f32),
                np.tile(np.sin(fr), (4, 1)).astype(f32))

    cos_k, sin_k = tab(np.arange(LWIN))
    common = {
        "wq_p": pack_qk(wq),
        "wk_p": pack_qk(wk),
        "wv_p": _tileize(wv.T.astype(BF), None),
        "wo_p": _tileize(np.asarray(w_out, f32).T.astype(BF), None),
        "wgu_p": wgu_p,
        "wd_p": wd_p,
        "rope_p": P.astype(BF),
        "ones_b": np.ones((128, 128), BF),
        "cos_k": cos_k,
        "sin_k": sin_k,
        "g1": np.ascontiguousarray(np.asarray(g_norm1, f32).reshape(NT, 128).T),
        "g2": np.ascontiguousarray(np.asarray(g_norm2, f32).reshape(NT, 128).T),
    }
    in_maps = []
    for c in range(N_CORES):
        b, qh = c // 2, c % 2
        cos_q, sin_q = tab(np.arange(qh * LQ, (qh + 1) * LQ))
        m = dict(common)
        xq = x[b, qh * LQ:(qh + 1) * LQ].T               # [1024, 512]
        m["xw_b"] = _tileize(x[b, :LWIN].T.astype(BF), None)
        m["xq_b"] = _tileize(xq.astype(BF), None)
        m["xq_f"] = _tileize(xq, None)
        m["xr_b"] = _tileize(
            x[b, LWIN + qh * REST:LWIN + (qh + 1) * REST].T.astype(BF), None)
        m["cos_q"] = cos_q
        m["sin_q"] = sin_q
        in_maps.append(m)
    return in_maps


def assemble_output(results):
    out = np.empty((4, 4096, WIDTH), np.float32)
    for c in range(N_CORES):
        b, qh = c // 2, c % 2
        o = results[c]["out_t"].reshape(128, NT, TOUT)
        ot = o.transpose(2, 1, 0).reshape(TOUT, WIDTH)   # [token, width]
        out[b, qh * LQ:(qh + 1) * LQ] = ot[:LQ]
        out[b, LWIN + qh * REST:LWIN + (qh + 1) * REST] = ot[LQ:]
    return out


_CACHE = {}


def kernel(x, w_qkv, w_out, g_norm1, g_norm2, w_gate, w_up, w_down):
    if "nc" not in _CACHE:
        _CACHE["nc"] = build_program()
    nc = _CACHE["nc"]
    in_maps = make_in_maps(x, w_qkv, w_out, g_norm1, g_norm2, w_gate, w_up, w_down)
    res = run_bass_kernel_spmd(nc, in_maps, list(range(N_CORES))).results
    return assemble_output(res)
